# revision 28
# baseline (speedup 1.0000x reference)
"""GAT (2-layer, 6-head) + GraphNorm + readout MLP on 8 Trainium2 cores.

Sharding: graph-level data parallelism. 48 fixed-size graphs (228 nodes,
7296 edges each, edges never cross graphs) -> 6 graphs per core.
Weights replicated. Per-core device kernel does all model math
(dense per-graph attention: the edge list is converted to a per-graph
multiplicity matrix M[s,d] on the host, a pure integer relayout of
edge_index; self-loops = +I).

kernel(**inputs) -> np.ndarray [48, 2] float32.
"""
import sys
sys.path.insert(0, '/opt/trn_rl_repo')

import numpy as np

import concourse.bass as bass
import concourse.bacc as bacc
import concourse.mybir as mybir
import concourse.tile as tile
from concourse import masks
from concourse import bass_utils

F32 = mybir.dt.float32
BF16 = mybir.dt.bfloat16
Alu = mybir.AluOpType
Act = mybir.ActivationFunctionType

H, C = 6, 64
HC = 384
NPG = 228          # nodes per graph
B = 48             # graphs
GPC = 6            # graphs per core
NCORES = 8
DEG = 32
EPG = NPG * DEG    # listed edges per graph
F_IN = 228
NH = 114           # node half-chunk
NCLS = 2
K1 = NPG * HC      # 87552 lin1 contraction
NJ = K1 // NH      # 768 lin1 k-chunks
LIN1_GRP = 16      # j-chunks per streamed lin1 weight tile

_last_results = {"exec_time_ns": None}


def _ensure_axon_hooks():
    """Make BASS_TRACE-driven NTFF profiling under axon degrade gracefully.

    The image's `antenv` lacks `axon_hooks`; concourse imports it
    unconditionally when trace is requested. Provide the module (and
    register the real ctypes hook when available)."""
    try:
        import antenv.axon_hooks  # noqa: F401
        return
    except ImportError:
        pass
    import types
    try:
        import antenv
    except ImportError:
        return
    mod = types.ModuleType("antenv.axon_hooks")
    holder = {"hook": None}
    mod.set_axon_ntff_profile_hook = lambda h: holder.__setitem__("hook", h)
    mod.get_axon_ntff_profile_hook = lambda: holder["hook"]
    sys.modules["antenv.axon_hooks"] = mod
    antenv.axon_hooks = mod
    try:
        from trn_agent_boot.trn_boot import _ntff_profile_via_ctypes
        hook = _ntff_profile_via_ctypes('/opt/axon/libaxon_pjrt.so')
        if hook is not None:
            mod.set_axon_ntff_profile_hook(hook)
    except Exception:
        pass
    # uploads to the artifact bucket are unavailable in this container
    _orig_upload = bass_utils.upload_artifacts

    def _safe_upload(tmpdir):
        try:
            return _orig_upload(tmpdir)
        except Exception:
            return "local://" + str(tmpdir)

    bass_utils.upload_artifacts = _safe_upload


_ensure_axon_hooks()


def _build_program():
    nc = bacc.Bacc("TRN2", target_bir_lowering=False, debug=False)

    dt_in = {}

    def din(name, shape, dtype=F32):
        t = nc.dram_tensor(name, shape, dtype, kind="ExternalInput")
        dt_in[name] = t
        return t

    xT_d = din("xT", [GPC, F_IN, NPG])           # per-graph x transposed [f, n]
    M_d = din("Mm", [GPC, NPG, NPG], BF16)       # multiplicity+selfloop [s, d]
    w1_d = din("w1", [F_IN, HC])
    w2_d = din("w2", [HC, HC])
    asad1_d = din("asad1", [HC, 12])
    asad2_d = din("asad2", [HC, 12])
    b1_d = din("b1c", [HC, 1])
    b2_d = din("b2c", [HC, 1])
    gn1_d = din("gn1", [HC, 3])                  # cols: w, b, ms
    gn2_d = din("gn2", [HC, 3])
    lin1_d = din("lin1r", [NH, NJ * 64], BF16)   # reordered lin1_w (see host)
    lin1b_d = din("lin1b", [64, 1])
    bn_d = din("bn", [64, 2])                    # cols: scale, shift (folded eval BN)
    lin2w_d = din("lin2w", [64, NCLS])
    lin2b_d = din("lin2b", [NCLS, 1])

    out_d = nc.dram_tensor("out", [NCLS, GPC], F32, kind="ExternalOutput")

    with tile.TileContext(nc) as tc:
        _emit(tc, dt_in, out_d)

    nc.finalize()
    return nc


import os
_STAGE_ORDER = ["h", "flat", "hA", "z", "agg", "post", "layer2", "x3a", "lin1", "full"]
_STAGE = os.environ.get("GAT_STAGE", "full")


def _stage_ge(s):
    return _STAGE_ORDER.index(_STAGE) >= _STAGE_ORDER.index(s)


def _emit(tc, din, out_d):
    nc = tc.nc
    ctxs = []

    cst = tc.alloc_tile_pool(name="cst", bufs=1)
    gio = tc.alloc_tile_pool(name="gio", bufs=2)
    wk = tc.alloc_tile_pool(name="wk", bufs=3)
    lwp = tc.alloc_tile_pool(name="lwp", bufs=3)
    psA = tc.alloc_tile_pool(name="psA", bufs=3, space="PSUM")
    psB = tc.alloc_tile_pool(name="psB", bufs=2, space="PSUM")
    psC = tc.alloc_tile_pool(name="psC", bufs=1, space="PSUM")

    # ---- constants / weights in SBUF ----
    ident = cst.tile([128, 128], F32)
    masks.make_identity(nc, ident[:])
    onesrow = cst.tile([1, 456], F32)
    nc.vector.memset(onesrow[:], 1.0)
    epsc = cst.tile([128, 1], F32)
    nc.vector.memset(epsc[:], 1e-5)

    w1 = cst.tile([NH, 2 * HC], F32)        # f-chunk fc at cols [fc*HC, +HC]
    for fc in range(2):
        nc.sync.dma_start(w1[:, fc * HC:(fc + 1) * HC], din["w1"].ap()[fc * NH:(fc + 1) * NH, :])
    w2 = cst.tile([128, 3 * HC], F32)
    for kc in range(3):
        nc.sync.dma_start(w2[:, kc * HC:(kc + 1) * HC], din["w2"].ap()[kc * 128:(kc + 1) * 128, :])
    asad1 = cst.tile([128, 3 * 12], F32)
    asad2 = cst.tile([128, 3 * 12], F32)
    for kc in range(3):
        nc.sync.dma_start(asad1[:, kc * 12:(kc + 1) * 12], din["asad1"].ap()[kc * 128:(kc + 1) * 128, :])
        nc.sync.dma_start(asad2[:, kc * 12:(kc + 1) * 12], din["asad2"].ap()[kc * 128:(kc + 1) * 128, :])
    bias_gn = cst.tile([128, 3 * 8], F32)   # per c-chunk: cols [b, gnw, gnb, gnms]x2 layers
    for kc in range(3):
        s = kc * 8
        nc.sync.dma_start(bias_gn[:, s:s + 1], din["b1c"].ap()[kc * 128:(kc + 1) * 128, :])
        nc.sync.dma_start(bias_gn[:, s + 1:s + 4], din["gn1"].ap()[kc * 128:(kc + 1) * 128, :])
        nc.sync.dma_start(bias_gn[:, s + 4:s + 5], din["b2c"].ap()[kc * 128:(kc + 1) * 128, :])
        nc.sync.dma_start(bias_gn[:, s + 5:s + 8], din["gn2"].ap()[kc * 128:(kc + 1) * 128, :])
    head64 = cst.tile([64, 8], F32)         # lin1b, bn scale, bn shift
    nc.sync.dma_start(head64[:, 0:1], din["lin1b"].ap()[:, :])
    nc.sync.dma_start(head64[:, 1:3], din["bn"].ap()[:, :])
    lin2w = cst.tile([64, NCLS], F32)
    nc.sync.dma_start(lin2w[:], din["lin2w"].ap()[:, :])
    lin2b = cst.tile([NCLS, 1], F32)
    nc.sync.dma_start(lin2b[:], din["lin2b"].ap()[:, :])

    X3A = cst.tile([NH, 2 * GPC * HC], BF16)   # [nch][g][c] flattened

    # bf16 casts of matmul operands (done once, on device)
    w1b = cst.tile([NH, 2 * HC], BF16)
    nc.vector.tensor_copy(w1b[:], w1[:])
    w2b = cst.tile([128, 3 * HC], BF16)
    nc.vector.tensor_copy(w2b[:], w2[:])
    asad1b = cst.tile([128, 3 * 12], BF16)
    nc.vector.tensor_copy(asad1b[:], asad1[:])
    asad2b = cst.tile([128, 3 * 12], BF16)
    nc.vector.tensor_copy(asad2b[:], asad2[:])
    onesrow_b = cst.tile([1, 456], BF16)
    nc.vector.memset(onesrow_b[:], 1.0)
    identb = cst.tile([128, 128], BF16)
    masks.make_identity(nc, identb[:])

    def layer(g, inB, Mg, nkc, wts, asadt, lay):
        """One GAT layer + bias + elu + graphnorm.
        inB: sb tile [p, nkc*NPG] channel-major input (k-chunk kc at cols kc*NPG).
        nkc: #k-chunks (2 for layer1 with p=114, 3 for layer2 with p=128).
        wts: weight tile [p, nkc*HC]; asadt: [128, 3*12].
        Returns x2B sb tile [128, 3*NPG]."""
        # h = W.T @ x  -> hB [c(3x128), n]
        hB = wk.tile([128, 3 * NPG], BF16, tag="hB")
        for ck in range(3):
            h_ps = psA.tile([128, NPG], F32, tag="ps1")
            for kc in range(nkc):
                nc.tensor.matmul(h_ps[:],
                                 wts[:, kc * HC + ck * 128: kc * HC + (ck + 1) * 128],
                                 inB[:, kc * NPG:(kc + 1) * NPG],
                                 start=(kc == 0), stop=(kc == nkc - 1))
            nc.scalar.copy(hB[:, ck * NPG:(ck + 1) * NPG], h_ps[:])

        def dummy():
            d = wk.tile([128, 3 * NPG], F32, tag="x2B")
            nc.vector.memset(d[:], 0.0)
            return d
        if not _stage_ge("flat"):
            return dummy()
        # a12T [12, n] = asad.T @ hB
        a12_ps = psA.tile([12, NPG], F32, tag="ps1")
        for ck in range(3):
            nc.tensor.matmul(a12_ps[:], asadt[:, ck * 12:(ck + 1) * 12],
                             hB[:, ck * NPG:(ck + 1) * NPG],
                             start=(ck == 0), stop=(ck == 2))
        a12T = wk.tile([12, NPG], F32, tag="a12T")
        nc.vector.tensor_copy(a12T[:], a12_ps[:])
        # flatten to one row [1, 12*228]
        a12f = wk.tile([1, 12 * NPG], F32, tag="a12f")
        nc.sync.dma_start(a12f[:], a12T[:])
        if not _stage_ge("hA"):
            return dummy()

        # hA65 [sc][114, 6*65]: transposed h + ones column per head
        hA65 = wk.tile([NH, 2 * 390], BF16, tag="hA65")
        for sc in range(2):
            for ck in range(3):
                tp = psB.tile([NH, 128], BF16, tag="tpb")
                nc.tensor.transpose(tp[:], hB[:, ck * NPG + sc * NH: ck * NPG + sc * NH + NH],
                                    identb[:])
                dst = hA65[:, sc * 390 + ck * 130: sc * 390 + (ck + 1) * 130]
                nc.scalar.copy(dst.rearrange("p (h c) -> p h c", h=2)[:, :, 0:64],
                               tp[:].rearrange("p (h c) -> p h c", h=2))
            on = hA65[:, sc * 390:(sc + 1) * 390].rearrange("p (h c) -> p h c", c=65)
            nc.vector.memset(on[:, :, 64:65], 1.0)

        if not _stage_ge("z"):
            return dummy()
        # attention weights W [sc][114, 6*228] = exp(lrelu(a1[s]+a2[d])) * M
        Wt = wk.tile([NH, 2 * 1368], BF16, tag="Wt")
        for sc in range(2):
            for hp in range(3):
                h0 = 2 * hp
                z = psA.tile([NH, 456], F32, tag="ps1")
                nc.tensor.matmul(z[:], onesrow[0:1, 0:NH],
                                 a12f[0:1, (6 + h0) * NPG: (6 + h0) * NPG + 456],
                                 start=True, stop=False)
                nc.tensor.matmul(z[:, 0:NPG],
                                 a12f[0:1, h0 * NPG + sc * NH: h0 * NPG + sc * NH + NH],
                                 onesrow[0:1, 0:NPG], start=False, stop=False)
                nc.tensor.matmul(z[:, NPG:456],
                                 a12f[0:1, (h0 + 1) * NPG + sc * NH: (h0 + 1) * NPG + sc * NH + NH],
                                 onesrow[0:1, 0:NPG], start=False, stop=True)
                u = wk.tile([NH, 456], F32, tag="u")
                nc.scalar.activation(u[:], z[:], Act.Identity, scale=0.2)
                t = wk.tile([NH, 456], F32, tag="t")
                nc.vector.tensor_tensor(out=t[:], in0=z[:], in1=u[:], op=Alu.max)
                e = wk.tile([NH, 456], BF16, tag="e")
                nc.scalar.activation(e[:], t[:], Act.Exp)
                m_b = Mg[:, sc * NPG:(sc + 1) * NPG] \
                    .rearrange("p (a d) -> p a d", a=1).broadcast_to((NH, 2, NPG))
                nc.vector.tensor_tensor(
                    out=Wt[:, sc * 1368 + hp * 456: sc * 1368 + (hp + 1) * 456]
                        .rearrange("p (a d) -> p a d", a=2),
                    in0=e[:].rearrange("p (a d) -> p a d", a=2),
                    in1=m_b, op=Alu.mult)

        if not _stage_ge("agg"):
            return dummy()
        # aggregation: numer65[dc][114, 6*65]
        out1A = wk.tile([NH, 2 * HC], F32, tag="out1A")
        for dc in range(2):
            num_ps = psA.tile([NH, 390], F32, tag="ps1")
            for h in range(6):
                for sc in range(2):
                    nc.tensor.matmul(
                        num_ps[:, h * 65:(h + 1) * 65],
                        Wt[:, sc * 1368 + h * NPG + dc * NH: sc * 1368 + h * NPG + dc * NH + NH],
                        hA65[:, sc * 390 + h * 65: sc * 390 + (h + 1) * 65],
                        start=(sc == 0), stop=(sc == 1))
            rec = wk.tile([NH, 6], F32, tag="rec")
            nc.vector.reciprocal(rec[:], num_ps[:].rearrange("p (h c) -> p h c", c=65)[:, :, 64:65]
                                 .rearrange("p h c -> p (h c)"))
            nc.vector.tensor_tensor(
                out=out1A[:, dc * HC:(dc + 1) * HC].rearrange("p (h c) -> p h c", h=6),
                in0=num_ps[:].rearrange("p (h c) -> p h c", c=65)[:, :, 0:64],
                in1=rec[:].rearrange("p (h c) -> p h c", c=1).broadcast_to((NH, 6, 64)),
                op=Alu.mult)

        if not _stage_ge("post"):
            return dummy()
        # transpose to channel-major, +bias, elu, graphnorm
        x2B = wk.tile([128, 3 * NPG], F32, tag="x2B")
        bofs = 0 if lay == 0 else 4
        for ck in range(3):
            xb = wk.tile([128, NPG], F32, tag="xb")
            for dc in range(2):
                tp2 = psB.tile([128, NH], F32, tag="tp")
                nc.tensor.transpose(tp2[:], out1A[:, dc * HC + ck * 128: dc * HC + (ck + 1) * 128],
                                    ident[0:NH, 0:NH])
                nc.vector.tensor_scalar_add(xb[:, dc * NH:(dc + 1) * NH], tp2[:],
                                            bias_gn[:, ck * 8 + bofs: ck * 8 + bofs + 1])
            mm = wk.tile([128, NPG], F32, tag="mm")
            nc.vector.tensor_scalar_min(mm[:], xb[:], 0.0)
            ee = wk.tile([128, NPG], F32, tag="ee")
            nc.scalar.activation(ee[:], mm[:], Act.Exp)
            xe = wk.tile([128, NPG], F32, tag="xe")
            nc.vector.scalar_tensor_tensor(xe[:], ee[:], -1.0, xb[:],
                                           op0=Alu.add, op1=Alu.max)
            # graphnorm
            red = wk.tile([128, 1], F32, tag="red")
            nc.vector.tensor_reduce(red[:], xe[:], axis=mybir.AxisListType.X, op=Alu.add)
            tmp = wk.tile([128, 1], F32, tag="tmp")
            nc.vector.scalar_tensor_tensor(tmp[:], red[:], 1.0 / NPG,
                                           bias_gn[:, ck * 8 + bofs + 3: ck * 8 + bofs + 4],
                                           op0=Alu.mult, op1=Alu.mult)
            xc = wk.tile([128, NPG], F32, tag="xc")
            nc.vector.tensor_scalar_sub(xc[:], xe[:], tmp[:])
            sq = wk.tile([128, NPG], F32, tag="sq")
            nc.vector.tensor_tensor(out=sq[:], in0=xc[:], in1=xc[:], op=Alu.mult)
            var = wk.tile([128, 1], F32, tag="var")
            nc.vector.tensor_reduce(var[:], sq[:], axis=mybir.AxisListType.X, op=Alu.add)
            vb = wk.tile([128, 1], F32, tag="vb")
            nc.vector.tensor_scalar(vb[:], var[:], 1.0 / NPG, 1e-5,
                                    op0=Alu.mult, op1=Alu.add)
            sd = wk.tile([128, 1], F32, tag="sd")
            nc.scalar.activation(sd[:], vb[:], Act.Sqrt)
            inv = wk.tile([128, 1], F32, tag="inv")
            nc.vector.reciprocal(inv[:], sd[:])
            is2 = wk.tile([128, 1], F32, tag="is2")
            nc.vector.tensor_tensor(out=is2[:], in0=inv[:],
                                    in1=bias_gn[:, ck * 8 + bofs + 1: ck * 8 + bofs + 2],
                                    op=Alu.mult)
            nc.vector.scalar_tensor_tensor(
                x2B[:, ck * NPG:(ck + 1) * NPG], xc[:], is2[:],
                bias_gn[:, ck * 8 + bofs + 2: ck * 8 + bofs + 3].broadcast_to((128, NPG)),
                op0=Alu.mult, op1=Alu.add)
        return x2B

    for g in range(GPC):
        xT = gio.tile([NH, 2 * NPG], F32, tag="xT")
        for fc in range(2):
            nc.sync.dma_start(xT[:, fc * NPG:(fc + 1) * NPG],
                              din["xT"].ap()[g, fc * NH:(fc + 1) * NH, :])
        inB_M = gio.tile([NH, 2 * NPG], BF16, tag="Mg")
        for sc in range(2):
            nc.sync.dma_start(inB_M[:, sc * NPG:(sc + 1) * NPG],
                              din["Mm"].ap()[g, sc * NH:(sc + 1) * NH, :])
        xTb = gio.tile([NH, 2 * NPG], BF16, tag="xTb")
        nc.vector.tensor_copy(xTb[:], xT[:])

        x2B = layer(g, xTb, inB_M, 2, w1b, asad1b, 0)
        x2Bb = wk.tile([128, 3 * NPG], BF16, tag="x2Bb")
        nc.vector.tensor_copy(x2Bb[:], x2B[:])
        x3B = layer(g, x2Bb, inB_M, 3, w2b, asad2b, 1) if _stage_ge("layer2") else x2B
        if not _stage_ge("x3a"):
            continue

        # pack into X3A (node-major transpose, bf16)
        for ck in range(3):
            for nch in range(2):
                tp3 = psB.tile([NH, 128], F32, tag="tp")
                nc.tensor.transpose(tp3[:], x3B[:, ck * NPG + nch * NH: ck * NPG + nch * NH + NH],
                                    ident[:])
                nc.scalar.copy(
                    X3A[:, nch * GPC * HC + g * HC + ck * 128: nch * GPC * HC + g * HC + (ck + 1) * 128],
                    tp3[:])

    # ---- readout ----
    if not _stage_ge("lin1"):
        ob0 = wk.tile([NCLS, GPC], F32, tag="ob")
        nc.vector.memset(ob0[:], 0.0)
        nc.sync.dma_start(out_d.ap()[:, :], ob0[:])
        for p in (psC, psB, psA, lwp, wk, gio, cst):
            p.release()
        return
    y_ps = psC.tile([64, GPC], F32, tag="y")
    for jg in range(NJ // LIN1_GRP):
        lw = lwp.tile([NH, LIN1_GRP * 64], BF16, tag="lw")
        nc.sync.dma_start(lw[:], din["lin1r"].ap()[:, jg * LIN1_GRP * 64:(jg + 1) * LIN1_GRP * 64])
        for jj in range(LIN1_GRP):
            j = jg * LIN1_GRP + jj
            c, nch = j // 2, j % 2
            rhs = X3A[:, nch * GPC * HC:(nch + 1) * GPC * HC] \
                .rearrange("p (g c) -> p g c", c=HC)[:, :, c:c + 1] \
                .rearrange("p g c -> p (g c)")
            nc.tensor.matmul(y_ps[:], lw[:, jj * 64:(jj + 1) * 64], rhs,
                             start=(j == 0), stop=(j == NJ - 1))

    yb = wk.tile([64, GPC], F32, tag="yb")
    nc.vector.tensor_scalar_add(yb[:], y_ps[:], head64[:, 0:1])
    m2 = wk.tile([64, GPC], F32, tag="m2")
    nc.vector.tensor_scalar_min(m2[:], yb[:], 0.0)
    e2 = wk.tile([64, GPC], F32, tag="e2")
    nc.scalar.activation(e2[:], m2[:], Act.Exp)
    ye = wk.tile([64, GPC], F32, tag="ye")
    nc.vector.scalar_tensor_tensor(ye[:], e2[:], -1.0, yb[:], op0=Alu.add, op1=Alu.max)
    # batchnorm (eval, host-folded): yn = ye * scale + shift
    yn = wk.tile([64, GPC], F32, tag="yn")
    nc.vector.scalar_tensor_tensor(yn[:], ye[:], head64[:, 1:2],
                                   head64[:, 2:3].broadcast_to((64, GPC)),
                                   op0=Alu.mult, op1=Alu.add)
    # lin2
    o_ps = psC.tile([NCLS, GPC], F32, tag="y")
    nc.tensor.matmul(o_ps[:], lin2w[:], yn[:], start=True, stop=True)
    ob = wk.tile([NCLS, GPC], F32, tag="ob")
    nc.vector.tensor_scalar_add(ob[:], o_ps[:], lin2b[:])
    nc.sync.dma_start(out_d.ap()[:, :], ob[:])

    for p in (psC, psB, psA, lwp, wk, gio, cst):
        p.release()


def _host_prep(inputs):
    """Build per-core input maps (pure sharding / integer relayout / dtype prep)."""
    x = np.asarray(inputs["x"], np.float32)
    ei = np.asarray(inputs["edge_index"])
    src, dst = np.asarray(ei[0], np.int64), np.asarray(ei[1], np.int64)

    # multiplicity matrices M[g, s, d] (+ self loops)
    g_of = src // NPG
    sl = src - g_of * NPG
    dl = dst - (dst // NPG) * NPG
    flat = g_of * (NPG * NPG) + sl * NPG + dl
    Mall = np.bincount(flat, minlength=B * NPG * NPG).astype(np.float32).reshape(B, NPG, NPG)
    Mall[:, np.arange(NPG), np.arange(NPG)] += 1.0
    import ml_dtypes
    Mall = Mall.astype(ml_dtypes.bfloat16)

    xg = x.reshape(B, NPG, F_IN)
    xT = np.ascontiguousarray(xg.transpose(0, 2, 1))  # [B, f, n]

    def mk_asad(a_s, a_d):
        a_s = np.asarray(a_s, np.float32)
        a_d = np.asarray(a_d, np.float32)
        out = np.zeros((HC, 12), np.float32)
        for h in range(H):
            out[h * C:(h + 1) * C, h] = a_s[h]
            out[h * C:(h + 1) * C, 6 + h] = a_d[h]
        return out

    gn1 = np.stack([np.asarray(inputs["gn1_w"], np.float32),
                    np.asarray(inputs["gn1_b"], np.float32),
                    np.asarray(inputs["gn1_ms"], np.float32)], axis=1)
    gn2 = np.stack([np.asarray(inputs["gn2_w"], np.float32),
                    np.asarray(inputs["gn2_b"], np.float32),
                    np.asarray(inputs["gn2_ms"], np.float32)], axis=1)
    bn_w = np.asarray(inputs["bn_w"], np.float64)
    bn_b = np.asarray(inputs["bn_b"], np.float64)
    bn_rm = np.asarray(inputs["bn_rm"], np.float64)
    bn_rv = np.asarray(inputs["bn_rv"], np.float64)
    bn_sc = bn_w / np.sqrt(bn_rv + 1e-5)
    bn_sh = bn_b - bn_rm * bn_sc
    bn = np.stack([bn_sc, bn_sh], axis=1).astype(np.float32)

    # lin1 reorder: rows (n=nch*114+p)*384+c -> [p, (c, nch, o)]
    import ml_dtypes
    lw = np.asarray(inputs["lin1_w"], np.float32).reshape(2, NH, HC, 64)
    lin1r = np.ascontiguousarray(lw.transpose(1, 2, 0, 3)).reshape(NH, NJ * 64) \
        .astype(ml_dtypes.bfloat16)

    shared = dict(
        w1=np.asarray(inputs["w1"], np.float32),
        w2=np.asarray(inputs["w2"], np.float32),
        asad1=mk_asad(inputs["as1"], inputs["ad1"]),
        asad2=mk_asad(inputs["as2"], inputs["ad2"]),
        b1c=np.asarray(inputs["b1"], np.float32).reshape(HC, 1),
        b2c=np.asarray(inputs["b2"], np.float32).reshape(HC, 1),
        gn1=gn1, gn2=gn2,
        lin1r=lin1r,
        lin1b=np.asarray(inputs["lin1_b"], np.float32).reshape(64, 1),
        bn=bn,
        lin2w=np.asarray(inputs["lin2_w"], np.float32),
        lin2b=np.asarray(inputs["lin2_b"], np.float32).reshape(NCLS, 1),
    )
    in_maps = []
    for core in range(NCORES):
        gs = slice(core * GPC, (core + 1) * GPC)
        m = dict(shared)
        m["xT"] = np.ascontiguousarray(xT[gs])
        m["Mm"] = np.ascontiguousarray(Mall[gs])
        in_maps.append(m)
    return in_maps


_cached_nc = None


def kernel(**inputs):
    global _cached_nc
    in_maps = _host_prep(inputs)
    if _cached_nc is None:
        _cached_nc = _build_program()
    nc = _cached_nc
    res = bass_utils.run_bass_kernel_spmd(nc, in_maps, core_ids=list(range(NCORES)))
    _last_results["exec_time_ns"] = res.exec_time_ns
    _last_results["res"] = res
    out = np.zeros((B, NCLS), np.float32)
    for core in range(NCORES):
        o = res.results[core]["out"]          # [2, 6]
        out[core * GPC:(core + 1) * GPC, :] = o.T
    return out


# revision 29
# speedup vs baseline: 1.2616x; 1.2616x over previous
"""GAT (2-layer, 6-head) + GraphNorm + readout MLP on 8 Trainium2 cores.

Sharding: graph-level data parallelism. 48 fixed-size graphs (228 nodes,
7296 edges each, edges never cross graphs) -> 6 graphs per core.
Weights replicated. Per-core device kernel does all model math
(dense per-graph attention: the edge list is converted to a per-graph
multiplicity matrix M[s,d] on the host, a pure integer relayout of
edge_index; self-loops = +I).

kernel(**inputs) -> np.ndarray [48, 2] float32.
"""
import sys
sys.path.insert(0, '/opt/trn_rl_repo')

import numpy as np

import concourse.bass as bass
import concourse.bacc as bacc
import concourse.mybir as mybir
import concourse.tile as tile
from concourse import masks
from concourse import bass_utils

F32 = mybir.dt.float32
BF16 = mybir.dt.bfloat16
Alu = mybir.AluOpType
Act = mybir.ActivationFunctionType

H, C = 6, 64
HC = 384
NPG = 228          # nodes per graph
B = 48             # graphs
GPC = 6            # graphs per core
NCORES = 8
DEG = 32
EPG = NPG * DEG    # listed edges per graph
F_IN = 228
NH = 114           # node half-chunk
NCLS = 2
K1 = NPG * HC      # 87552 lin1 contraction
NJ = K1 // NH      # 768 lin1 k-chunks
LIN1_GRP = 16      # j-chunks per streamed lin1 weight tile

_last_results = {"exec_time_ns": None}


def _ensure_axon_hooks():
    """Make BASS_TRACE-driven NTFF profiling under axon degrade gracefully.

    The image's `antenv` lacks `axon_hooks`; concourse imports it
    unconditionally when trace is requested. Provide the module (and
    register the real ctypes hook when available)."""
    try:
        import antenv.axon_hooks  # noqa: F401
        return
    except ImportError:
        pass
    import types
    try:
        import antenv
    except ImportError:
        return
    mod = types.ModuleType("antenv.axon_hooks")
    holder = {"hook": None}
    mod.set_axon_ntff_profile_hook = lambda h: holder.__setitem__("hook", h)
    mod.get_axon_ntff_profile_hook = lambda: holder["hook"]
    sys.modules["antenv.axon_hooks"] = mod
    antenv.axon_hooks = mod
    try:
        from trn_agent_boot.trn_boot import _ntff_profile_via_ctypes
        hook = _ntff_profile_via_ctypes('/opt/axon/libaxon_pjrt.so')
        if hook is not None:
            mod.set_axon_ntff_profile_hook(hook)
    except Exception:
        pass
    # uploads to the artifact bucket are unavailable in this container
    _orig_upload = bass_utils.upload_artifacts

    def _safe_upload(tmpdir):
        try:
            return _orig_upload(tmpdir)
        except Exception:
            return "local://" + str(tmpdir)

    bass_utils.upload_artifacts = _safe_upload


_ensure_axon_hooks()


def _build_program():
    nc = bacc.Bacc("TRN2", target_bir_lowering=False, debug=False)

    dt_in = {}

    def din(name, shape, dtype=F32):
        t = nc.dram_tensor(name, shape, dtype, kind="ExternalInput")
        dt_in[name] = t
        return t

    xT_d = din("xT", [GPC, F_IN, NPG])           # per-graph x transposed [f, n]
    M_d = din("Mm", [GPC, NPG, NPG], BF16)       # multiplicity+selfloop [s, d]
    w1_d = din("w1", [F_IN, HC])
    w2_d = din("w2", [HC, HC])
    asad1_d = din("asad1", [HC, 12])
    asad2_d = din("asad2", [HC, 12])
    b1_d = din("b1c", [HC, 1])
    b2_d = din("b2c", [HC, 1])
    gn1_d = din("gn1", [HC, 3])                  # cols: w, b, ms
    gn2_d = din("gn2", [HC, 3])
    lin1_d = din("lin1r", [NH, NJ * 64], BF16)   # reordered lin1_w (see host)
    lin1b_d = din("lin1b", [64, 1])
    bn_d = din("bn", [64, 2])                    # cols: scale, shift (folded eval BN)
    lin2w_d = din("lin2w", [64, NCLS])
    lin2b_d = din("lin2b", [NCLS, 1])

    out_d = nc.dram_tensor("out", [NCLS, GPC], F32, kind="ExternalOutput")

    with tile.TileContext(nc) as tc:
        _emit(tc, dt_in, out_d)

    nc.finalize()
    return nc


import os
_STAGE_ORDER = ["h", "flat", "hA", "z", "agg", "post", "layer2", "x3a", "lin1", "full"]
_STAGE = os.environ.get("GAT_STAGE", "full")


def _stage_ge(s):
    return _STAGE_ORDER.index(_STAGE) >= _STAGE_ORDER.index(s)


def _emit(tc, din, out_d):
    nc = tc.nc
    ctxs = []

    cst = tc.alloc_tile_pool(name="cst", bufs=1)
    gio = tc.alloc_tile_pool(name="gio", bufs=2)
    wk = tc.alloc_tile_pool(name="wk", bufs=3)
    lwp = tc.alloc_tile_pool(name="lwp", bufs=3)
    psA = tc.alloc_tile_pool(name="psA", bufs=3, space="PSUM")
    psB = tc.alloc_tile_pool(name="psB", bufs=2, space="PSUM")
    psC = tc.alloc_tile_pool(name="psC", bufs=1, space="PSUM")

    # ---- constants / weights in SBUF ----
    ident = cst.tile([128, 128], F32)
    masks.make_identity(nc, ident[:])
    onesrow = cst.tile([1, 456], F32)
    nc.vector.memset(onesrow[:], 1.0)
    epsc = cst.tile([128, 1], F32)
    nc.vector.memset(epsc[:], 1e-5)

    w1 = cst.tile([NH, 2 * HC], F32)        # f-chunk fc at cols [fc*HC, +HC]
    for fc in range(2):
        nc.sync.dma_start(w1[:, fc * HC:(fc + 1) * HC], din["w1"].ap()[fc * NH:(fc + 1) * NH, :])
    w2 = cst.tile([128, 3 * HC], F32)
    for kc in range(3):
        nc.sync.dma_start(w2[:, kc * HC:(kc + 1) * HC], din["w2"].ap()[kc * 128:(kc + 1) * 128, :])
    asad1 = cst.tile([128, 3 * 12], F32)
    asad2 = cst.tile([128, 3 * 12], F32)
    for kc in range(3):
        nc.sync.dma_start(asad1[:, kc * 12:(kc + 1) * 12], din["asad1"].ap()[kc * 128:(kc + 1) * 128, :])
        nc.sync.dma_start(asad2[:, kc * 12:(kc + 1) * 12], din["asad2"].ap()[kc * 128:(kc + 1) * 128, :])
    bias_gn = cst.tile([128, 3 * 8], F32)   # per c-chunk: cols [b, gnw, gnb, gnms]x2 layers
    for kc in range(3):
        s = kc * 8
        nc.sync.dma_start(bias_gn[:, s:s + 1], din["b1c"].ap()[kc * 128:(kc + 1) * 128, :])
        nc.sync.dma_start(bias_gn[:, s + 1:s + 4], din["gn1"].ap()[kc * 128:(kc + 1) * 128, :])
        nc.sync.dma_start(bias_gn[:, s + 4:s + 5], din["b2c"].ap()[kc * 128:(kc + 1) * 128, :])
        nc.sync.dma_start(bias_gn[:, s + 5:s + 8], din["gn2"].ap()[kc * 128:(kc + 1) * 128, :])
    head64 = cst.tile([64, 8], F32)         # lin1b, bn scale, bn shift
    nc.sync.dma_start(head64[:, 0:1], din["lin1b"].ap()[:, :])
    nc.sync.dma_start(head64[:, 1:3], din["bn"].ap()[:, :])
    lin2w = cst.tile([64, NCLS], F32)
    nc.sync.dma_start(lin2w[:], din["lin2w"].ap()[:, :])
    lin2b = cst.tile([NCLS, 1], F32)
    nc.sync.dma_start(lin2b[:], din["lin2b"].ap()[:, :])

    X3A = cst.tile([NH, 2 * GPC * HC], BF16)   # [nch][g][c] flattened

    # bf16 casts of matmul operands (done once, on device)
    w1b = cst.tile([NH, 2 * HC], BF16)
    nc.vector.tensor_copy(w1b[:], w1[:])
    w2b = cst.tile([128, 3 * HC], BF16)
    nc.vector.tensor_copy(w2b[:], w2[:])
    asad1b = cst.tile([128, 3 * 12], BF16)
    nc.vector.tensor_copy(asad1b[:], asad1[:])
    asad2b = cst.tile([128, 3 * 12], BF16)
    nc.vector.tensor_copy(asad2b[:], asad2[:])
    onesrow_b = cst.tile([1, 456], BF16)
    nc.vector.memset(onesrow_b[:], 1.0)
    identb = cst.tile([128, 128], BF16)
    masks.make_identity(nc, identb[:])

    def layer(g, inB, Mg, nkc, wts, asadt, lay):
        """One GAT layer + bias + elu + graphnorm.
        inB: sb tile [p, nkc*NPG] channel-major input (k-chunk kc at cols kc*NPG).
        nkc: #k-chunks (2 for layer1 with p=114, 3 for layer2 with p=128).
        wts: weight tile [p, nkc*HC]; asadt: [128, 3*12].
        Returns x2B sb tile [128, 3*NPG]."""
        # h = W.T @ x  -> hB [c(3x128), n]
        hB = wk.tile([128, 3 * NPG], BF16, tag="hB")
        for ck in range(3):
            h_ps = psA.tile([128, NPG], F32, tag="ps1")
            for kc in range(nkc):
                nc.tensor.matmul(h_ps[:],
                                 wts[:, kc * HC + ck * 128: kc * HC + (ck + 1) * 128],
                                 inB[:, kc * NPG:(kc + 1) * NPG],
                                 start=(kc == 0), stop=(kc == nkc - 1))
            nc.scalar.copy(hB[:, ck * NPG:(ck + 1) * NPG], h_ps[:])

        def dummy():
            d = wk.tile([128, 3 * NPG], F32, tag="x2B")
            nc.vector.memset(d[:], 0.0)
            return d
        if not _stage_ge("flat"):
            return dummy()
        # a12T [12, n] = asad.T @ hB
        a12_ps = psA.tile([12, NPG], F32, tag="ps1")
        for ck in range(3):
            nc.tensor.matmul(a12_ps[:], asadt[:, ck * 12:(ck + 1) * 12],
                             hB[:, ck * NPG:(ck + 1) * NPG],
                             start=(ck == 0), stop=(ck == 2))
        a12T = wk.tile([12, NPG], BF16, tag="a12T")
        nc.vector.tensor_copy(a12T[:], a12_ps[:])
        # flatten to one row [1, 12*228]
        a12f = wk.tile([1, 12 * NPG], BF16, tag="a12f")
        nc.sync.dma_start(a12f[:], a12T[:])
        if not _stage_ge("hA"):
            return dummy()

        # hA65 [sc][114, 6*65]: transposed h + ones column per head
        hA65 = wk.tile([NH, 2 * 390], BF16, tag="hA65")
        for sc in range(2):
            for ck in range(3):
                tp = psB.tile([NH, 128], BF16, tag="tpb")
                nc.tensor.transpose(tp[:], hB[:, ck * NPG + sc * NH: ck * NPG + sc * NH + NH],
                                    identb[:])
                dst = hA65[:, sc * 390 + ck * 130: sc * 390 + (ck + 1) * 130]
                nc.scalar.copy(dst.rearrange("p (h c) -> p h c", h=2)[:, :, 0:64],
                               tp[:].rearrange("p (h c) -> p h c", h=2))
            on = hA65[:, sc * 390:(sc + 1) * 390].rearrange("p (h c) -> p h c", c=65)
            nc.vector.memset(on[:, :, 64:65], 1.0)

        if not _stage_ge("z"):
            return dummy()
        # attention weights W [sc][114, 6*228] = exp(lrelu(a1[s]+a2[d])) * M
        Wt = wk.tile([NH, 2 * 1368], BF16, tag="Wt")
        for sc in range(2):
            for hp in range(3):
                h0 = 2 * hp
                z = psA.tile([NH, 456], F32, tag="ps1")
                nc.tensor.matmul(z[:], onesrow_b[0:1, 0:NH],
                                 a12f[0:1, (6 + h0) * NPG: (6 + h0) * NPG + 456],
                                 start=True, stop=False)
                nc.tensor.matmul(z[:, 0:NPG],
                                 a12f[0:1, h0 * NPG + sc * NH: h0 * NPG + sc * NH + NH],
                                 onesrow_b[0:1, 0:NPG], start=False, stop=False)
                nc.tensor.matmul(z[:, NPG:456],
                                 a12f[0:1, (h0 + 1) * NPG + sc * NH: (h0 + 1) * NPG + sc * NH + NH],
                                 onesrow_b[0:1, 0:NPG], start=False, stop=True)
                u = wk.tile([NH, 456], F32, tag="u")
                nc.scalar.activation(u[:], z[:], Act.Identity, scale=0.2)
                t = wk.tile([NH, 456], F32, tag="t")
                nc.vector.tensor_tensor(out=t[:], in0=z[:], in1=u[:], op=Alu.max)
                e = wk.tile([NH, 456], BF16, tag="e")
                nc.scalar.activation(e[:], t[:], Act.Exp)
                m_b = Mg[:, sc * NPG:(sc + 1) * NPG] \
                    .rearrange("p (a d) -> p a d", a=1).broadcast_to((NH, 2, NPG))
                nc.vector.tensor_tensor(
                    out=Wt[:, sc * 1368 + hp * 456: sc * 1368 + (hp + 1) * 456]
                        .rearrange("p (a d) -> p a d", a=2),
                    in0=e[:].rearrange("p (a d) -> p a d", a=2),
                    in1=m_b, op=Alu.mult)

        if not _stage_ge("agg"):
            return dummy()
        # aggregation: numer65[dc][114, 6*65]
        out1A = wk.tile([NH, 2 * HC], F32, tag="out1A")
        for dc in range(2):
            num_ps = psA.tile([NH, 390], F32, tag="ps1")
            for h in range(6):
                for sc in range(2):
                    nc.tensor.matmul(
                        num_ps[:, h * 65:(h + 1) * 65],
                        Wt[:, sc * 1368 + h * NPG + dc * NH: sc * 1368 + h * NPG + dc * NH + NH],
                        hA65[:, sc * 390 + h * 65: sc * 390 + (h + 1) * 65],
                        start=(sc == 0), stop=(sc == 1))
            rec = wk.tile([NH, 6], F32, tag="rec")
            nc.vector.reciprocal(rec[:], num_ps[:].rearrange("p (h c) -> p h c", c=65)[:, :, 64:65]
                                 .rearrange("p h c -> p (h c)"))
            nc.vector.tensor_tensor(
                out=out1A[:, dc * HC:(dc + 1) * HC].rearrange("p (h c) -> p h c", h=6),
                in0=num_ps[:].rearrange("p (h c) -> p h c", c=65)[:, :, 0:64],
                in1=rec[:].rearrange("p (h c) -> p h c", c=1).broadcast_to((NH, 6, 64)),
                op=Alu.mult)

        if not _stage_ge("post"):
            return dummy()
        # transpose to channel-major, +bias, elu, graphnorm
        x2B = wk.tile([128, 3 * NPG], F32, tag="x2B")
        bofs = 0 if lay == 0 else 4
        for ck in range(3):
            xb = wk.tile([128, NPG], F32, tag="xb")
            for dc in range(2):
                tp2 = psB.tile([128, NH], F32, tag="tp")
                nc.tensor.transpose(tp2[:], out1A[:, dc * HC + ck * 128: dc * HC + (ck + 1) * 128],
                                    ident[0:NH, 0:NH])
                nc.vector.tensor_scalar_add(xb[:, dc * NH:(dc + 1) * NH], tp2[:],
                                            bias_gn[:, ck * 8 + bofs: ck * 8 + bofs + 1])
            mm = wk.tile([128, NPG], F32, tag="mm")
            nc.vector.tensor_scalar_min(mm[:], xb[:], 0.0)
            ee = wk.tile([128, NPG], F32, tag="ee")
            nc.scalar.activation(ee[:], mm[:], Act.Exp)
            xe = wk.tile([128, NPG], F32, tag="xe")
            nc.vector.scalar_tensor_tensor(xe[:], ee[:], -1.0, xb[:],
                                           op0=Alu.add, op1=Alu.max)
            # graphnorm
            red = wk.tile([128, 1], F32, tag="red")
            nc.vector.tensor_reduce(red[:], xe[:], axis=mybir.AxisListType.X, op=Alu.add)
            tmp = wk.tile([128, 1], F32, tag="tmp")
            nc.vector.scalar_tensor_tensor(tmp[:], red[:], 1.0 / NPG,
                                           bias_gn[:, ck * 8 + bofs + 3: ck * 8 + bofs + 4],
                                           op0=Alu.mult, op1=Alu.mult)
            xc = wk.tile([128, NPG], F32, tag="xc")
            nc.vector.tensor_scalar_sub(xc[:], xe[:], tmp[:])
            sq = wk.tile([128, NPG], F32, tag="sq")
            nc.vector.tensor_tensor(out=sq[:], in0=xc[:], in1=xc[:], op=Alu.mult)
            var = wk.tile([128, 1], F32, tag="var")
            nc.vector.tensor_reduce(var[:], sq[:], axis=mybir.AxisListType.X, op=Alu.add)
            vb = wk.tile([128, 1], F32, tag="vb")
            nc.vector.tensor_scalar(vb[:], var[:], 1.0 / NPG, 1e-5,
                                    op0=Alu.mult, op1=Alu.add)
            sd = wk.tile([128, 1], F32, tag="sd")
            nc.scalar.activation(sd[:], vb[:], Act.Sqrt)
            inv = wk.tile([128, 1], F32, tag="inv")
            nc.vector.reciprocal(inv[:], sd[:])
            is2 = wk.tile([128, 1], F32, tag="is2")
            nc.vector.tensor_tensor(out=is2[:], in0=inv[:],
                                    in1=bias_gn[:, ck * 8 + bofs + 1: ck * 8 + bofs + 2],
                                    op=Alu.mult)
            nc.vector.scalar_tensor_tensor(
                x2B[:, ck * NPG:(ck + 1) * NPG], xc[:], is2[:],
                bias_gn[:, ck * 8 + bofs + 2: ck * 8 + bofs + 3].broadcast_to((128, NPG)),
                op0=Alu.mult, op1=Alu.add)
        return x2B

    for g in range(GPC):
        xT = gio.tile([NH, 2 * NPG], F32, tag="xT")
        for fc in range(2):
            nc.sync.dma_start(xT[:, fc * NPG:(fc + 1) * NPG],
                              din["xT"].ap()[g, fc * NH:(fc + 1) * NH, :])
        inB_M = gio.tile([NH, 2 * NPG], BF16, tag="Mg")
        for sc in range(2):
            nc.sync.dma_start(inB_M[:, sc * NPG:(sc + 1) * NPG],
                              din["Mm"].ap()[g, sc * NH:(sc + 1) * NH, :])
        xTb = gio.tile([NH, 2 * NPG], BF16, tag="xTb")
        nc.vector.tensor_copy(xTb[:], xT[:])

        x2B = layer(g, xTb, inB_M, 2, w1b, asad1b, 0)
        x2Bb = wk.tile([128, 3 * NPG], BF16, tag="x2Bb")
        nc.vector.tensor_copy(x2Bb[:], x2B[:])
        x3B = layer(g, x2Bb, inB_M, 3, w2b, asad2b, 1) if _stage_ge("layer2") else x2B
        if not _stage_ge("x3a"):
            continue

        # pack into X3A (node-major transpose, bf16)
        for ck in range(3):
            for nch in range(2):
                tp3 = psB.tile([NH, 128], F32, tag="tp")
                nc.tensor.transpose(tp3[:], x3B[:, ck * NPG + nch * NH: ck * NPG + nch * NH + NH],
                                    ident[:])
                nc.scalar.copy(
                    X3A[:, nch * GPC * HC + g * HC + ck * 128: nch * GPC * HC + g * HC + (ck + 1) * 128],
                    tp3[:])

    # ---- readout ----
    if not _stage_ge("lin1"):
        ob0 = wk.tile([NCLS, GPC], F32, tag="ob")
        nc.vector.memset(ob0[:], 0.0)
        nc.sync.dma_start(out_d.ap()[:, :], ob0[:])
        for p in (psC, psB, psA, lwp, wk, gio, cst):
            p.release()
        return
    y_ps = psC.tile([64, GPC], F32, tag="y")
    for jg in range(NJ // LIN1_GRP):
        lw = lwp.tile([NH, LIN1_GRP * 64], BF16, tag="lw")
        nc.sync.dma_start(lw[:], din["lin1r"].ap()[:, jg * LIN1_GRP * 64:(jg + 1) * LIN1_GRP * 64])
        for jj in range(LIN1_GRP):
            j = jg * LIN1_GRP + jj
            c, nch = j // 2, j % 2
            rhs = X3A[:, nch * GPC * HC:(nch + 1) * GPC * HC] \
                .rearrange("p (g c) -> p g c", c=HC)[:, :, c:c + 1] \
                .rearrange("p g c -> p (g c)")
            nc.tensor.matmul(y_ps[:], lw[:, jj * 64:(jj + 1) * 64], rhs,
                             start=(j == 0), stop=(j == NJ - 1))

    yb = wk.tile([64, GPC], F32, tag="yb")
    nc.vector.tensor_scalar_add(yb[:], y_ps[:], head64[:, 0:1])
    m2 = wk.tile([64, GPC], F32, tag="m2")
    nc.vector.tensor_scalar_min(m2[:], yb[:], 0.0)
    e2 = wk.tile([64, GPC], F32, tag="e2")
    nc.scalar.activation(e2[:], m2[:], Act.Exp)
    ye = wk.tile([64, GPC], F32, tag="ye")
    nc.vector.scalar_tensor_tensor(ye[:], e2[:], -1.0, yb[:], op0=Alu.add, op1=Alu.max)
    # batchnorm (eval, host-folded): yn = ye * scale + shift
    yn = wk.tile([64, GPC], F32, tag="yn")
    nc.vector.scalar_tensor_tensor(yn[:], ye[:], head64[:, 1:2],
                                   head64[:, 2:3].broadcast_to((64, GPC)),
                                   op0=Alu.mult, op1=Alu.add)
    # lin2
    o_ps = psC.tile([NCLS, GPC], F32, tag="y")
    nc.tensor.matmul(o_ps[:], lin2w[:], yn[:], start=True, stop=True)
    ob = wk.tile([NCLS, GPC], F32, tag="ob")
    nc.vector.tensor_scalar_add(ob[:], o_ps[:], lin2b[:])
    nc.sync.dma_start(out_d.ap()[:, :], ob[:])

    for p in (psC, psB, psA, lwp, wk, gio, cst):
        p.release()


def _host_prep(inputs):
    """Build per-core input maps (pure sharding / integer relayout / dtype prep)."""
    x = np.asarray(inputs["x"], np.float32)
    ei = np.asarray(inputs["edge_index"])
    src, dst = np.asarray(ei[0], np.int64), np.asarray(ei[1], np.int64)

    # multiplicity matrices M[g, s, d] (+ self loops)
    g_of = src // NPG
    sl = src - g_of * NPG
    dl = dst - (dst // NPG) * NPG
    flat = g_of * (NPG * NPG) + sl * NPG + dl
    Mall = np.bincount(flat, minlength=B * NPG * NPG).astype(np.float32).reshape(B, NPG, NPG)
    Mall[:, np.arange(NPG), np.arange(NPG)] += 1.0
    import ml_dtypes
    Mall = Mall.astype(ml_dtypes.bfloat16)

    xg = x.reshape(B, NPG, F_IN)
    xT = np.ascontiguousarray(xg.transpose(0, 2, 1))  # [B, f, n]

    def mk_asad(a_s, a_d):
        a_s = np.asarray(a_s, np.float32)
        a_d = np.asarray(a_d, np.float32)
        out = np.zeros((HC, 12), np.float32)
        for h in range(H):
            out[h * C:(h + 1) * C, h] = a_s[h]
            out[h * C:(h + 1) * C, 6 + h] = a_d[h]
        return out

    gn1 = np.stack([np.asarray(inputs["gn1_w"], np.float32),
                    np.asarray(inputs["gn1_b"], np.float32),
                    np.asarray(inputs["gn1_ms"], np.float32)], axis=1)
    gn2 = np.stack([np.asarray(inputs["gn2_w"], np.float32),
                    np.asarray(inputs["gn2_b"], np.float32),
                    np.asarray(inputs["gn2_ms"], np.float32)], axis=1)
    bn_w = np.asarray(inputs["bn_w"], np.float64)
    bn_b = np.asarray(inputs["bn_b"], np.float64)
    bn_rm = np.asarray(inputs["bn_rm"], np.float64)
    bn_rv = np.asarray(inputs["bn_rv"], np.float64)
    bn_sc = bn_w / np.sqrt(bn_rv + 1e-5)
    bn_sh = bn_b - bn_rm * bn_sc
    bn = np.stack([bn_sc, bn_sh], axis=1).astype(np.float32)

    # lin1 reorder: rows (n=nch*114+p)*384+c -> [p, (c, nch, o)]
    import ml_dtypes
    lw = np.asarray(inputs["lin1_w"], np.float32).reshape(2, NH, HC, 64)
    lin1r = np.ascontiguousarray(lw.transpose(1, 2, 0, 3)).reshape(NH, NJ * 64) \
        .astype(ml_dtypes.bfloat16)

    shared = dict(
        w1=np.asarray(inputs["w1"], np.float32),
        w2=np.asarray(inputs["w2"], np.float32),
        asad1=mk_asad(inputs["as1"], inputs["ad1"]),
        asad2=mk_asad(inputs["as2"], inputs["ad2"]),
        b1c=np.asarray(inputs["b1"], np.float32).reshape(HC, 1),
        b2c=np.asarray(inputs["b2"], np.float32).reshape(HC, 1),
        gn1=gn1, gn2=gn2,
        lin1r=lin1r,
        lin1b=np.asarray(inputs["lin1_b"], np.float32).reshape(64, 1),
        bn=bn,
        lin2w=np.asarray(inputs["lin2_w"], np.float32),
        lin2b=np.asarray(inputs["lin2_b"], np.float32).reshape(NCLS, 1),
    )
    in_maps = []
    for core in range(NCORES):
        gs = slice(core * GPC, (core + 1) * GPC)
        m = dict(shared)
        m["xT"] = np.ascontiguousarray(xT[gs])
        m["Mm"] = np.ascontiguousarray(Mall[gs])
        in_maps.append(m)
    return in_maps


_cached_nc = None


def kernel(**inputs):
    global _cached_nc
    in_maps = _host_prep(inputs)
    if _cached_nc is None:
        _cached_nc = _build_program()
    nc = _cached_nc
    res = bass_utils.run_bass_kernel_spmd(nc, in_maps, core_ids=list(range(NCORES)))
    _last_results["exec_time_ns"] = res.exec_time_ns
    _last_results["res"] = res
    out = np.zeros((B, NCLS), np.float32)
    for core in range(NCORES):
        o = res.results[core]["out"]          # [2, 6]
        out[core * GPC:(core + 1) * GPC, :] = o.T
    return out
